# revision 2
# baseline (speedup 1.0000x reference)
"""Trainium2 Bass kernel for nn_AttnConvKernel (conv3x3 x2 -> unfold -> gram -> softmax).

Sharding: 8 cores = 4 batch samples x 2 H-halves. Each core computes both convs
for its half in a single fused matmul pass (x window stationary, [W1*scale|W2]
moving, f32r) that directly produces the [positions, channels] layout the
attention contraction needs; logits accumulate in PSUM across the 64 patch-row
tiles.

vs the earlier baseline: (1) x rows are DMA'd once into a 3-row ring buffer
(the 2 halo rows come from the next tile's buffer) instead of 5-row loads —
40% less HBM traffic; (2) the pairwise AllReduce is replaced by a
ReduceScatter with the scatter dim on the cout-halves — half the collective
traffic — and each pair-core transposes/softmaxes only its own 128 couts
(host concatenates); (3) the full body (loop + collective + tail) can be
replayed `reps` times inside one NEFF for robust differential timing."""

import numpy as np

B, CIN, COUT = 4, 128, 256
H = W = 384
WP = W // 3              # 128 patch columns
HALF_ROWS = H // 2       # 192
TILES = HALF_ROWS // 3   # 64 patch-rows per core
CH = CIN + COUT          # 384 fused output channels
COH = COUT // 2          # 128 couts per core in the split tail
NCORES = 8
SCALE = 1.0 / np.sqrt(CIN * 9)

_runners = {}


def _build_nc(reps=1, act_split=True, lag=3, psum4=False, act_every=2,
              chunks=2, ring=True, x_bufs=4, chunk_at=None, tail_split=False,
              x16=False):
    import concourse.mybir as mybir
    import concourse.tile as tile
    from concourse import bacc
    from concourse.masks import make_identity

    f32 = mybir.dt.float32
    f32r = mybir.dt.float32r
    xdt = mybir.dt.float16 if x16 else f32

    nc = bacc.Bacc(target_bir_lowering=False, num_devices=NCORES)
    x_half = nc.dram_tensor(
        "x_half", [CIN, HALF_ROWS + 2, W + 2], xdt, kind="ExternalInput"
    )
    wcat = nc.dram_tensor("wcat", [CIN, 9, CH], xdt, kind="ExternalInput")
    out_t = nc.dram_tensor("out", [COH, CIN, 9], f32, kind="ExternalOutput")
    # bank i in 0..3 holds k=2i,2i+1; bank 4 holds k=8
    if tail_split:
        assert chunks == 1
        bank_groups = [[0, 1], [2, 3, 4]]
    else:
        bank_groups = [[0, 1, 2, 3, 4]]

    def group_ks(banks):
        ks = []
        for i in banks:
            ks += [2 * i, 2 * i + 1] if i < 4 else [8]
        return ks

    cc_ins = [
        [
            nc.dram_tensor(f"cc_in{c}_{g}", [2, CIN, len(group_ks(banks)), COH], f32)
            for g, banks in enumerate(bank_groups)
        ]
        for c in range(chunks)
    ]
    cc_outs = [
        [
            nc.dram_tensor(f"cc_out{c}_{g}", [CIN, len(group_ks(banks)), COH], f32)
            for g, banks in enumerate(bank_groups)
        ]
        for c in range(chunks)
    ]

    if chunk_at is not None:
        assert chunks == 2
        bounds = [chunk_at, TILES]
    else:
        bounds = [round(TILES * (c + 1) / chunks) for c in range(chunks)]
    starts = {0} | {b for b in bounds[:-1]}
    ends = set(bounds)  # tile t is an end if t+1 in ends -> use t+1

    with tile.TileContext(nc) as tc:
        with (
            tc.tile_pool(name="xp", bufs=x_bufs) as xp,
            tc.tile_pool(name="wp", bufs=1) as wp,
            tc.tile_pool(
                name="yp", bufs=(9 if psum4 else 6) if lag <= 2 else lag + 5
            ) as yp,
            tc.tile_pool(name="sp", bufs=1) as sp,
            tc.tile_pool(name="pc", bufs=4 if psum4 else 3, space="PSUM") as pc,
            tc.tile_pool(name="pa", bufs=1, space="PSUM") as pa,
        ):
            wdt = mybir.dt.float16 if x16 else f32r
            w_sb = wp.tile([CIN, 9, CH], wdt)
            nc.sync.dma_start(
                out=w_sb,
                in_=wcat[:, :, :] if x16 else wcat[:, :, :].bitcast(f32r),
            )

            # persistent attn logit accumulators: 2 k's per PSUM bank
            attn_ps = [
                pa.tile([CIN, 2, COUT], f32, tag=f"attn{i}", name=f"attn{i}")
                for i in range(4)
            ]
            if psum4:
                acc8 = sp.tile([CIN, COUT], f32)
            else:
                attn_ps.append(
                    pa.tile([CIN, 1, COUT], f32, tag="attn4", name="attn4")
                )
            lgs = [
                [
                    sp.tile(
                        [CIN, len(group_ks(banks)) * COUT], f32, name=f"lg{c}_{g}"
                    )
                    for g, banks in enumerate(bank_groups)
                ]
                for c in range(chunks)
            ]

            def attn_mm(k, yk, t):
                chunk_start = t in starts
                chunk_end = (t + 1) in ends
                if psum4 and k == 8:
                    ps8 = pc.tile([CIN, COUT], f32, tag="conv", name="ps8")
                    nc.tensor.matmul(
                        ps8, yk[:, 0:CIN], yk[:, CIN:CH], start=True, stop=True
                    )
                    nc.vector.tensor_add(acc8, acc8, ps8)
                    return
                # start=True clears has_written for the WHOLE bank, so only the
                # first k of each 2-k bank may set it (at each chunk start).
                nc.tensor.matmul(
                    attn_ps[k // 2][:, k % 2, :],
                    yk[:, 0:CIN],
                    yk[:, CIN:CH],
                    start=(chunk_start and k % 2 == 0),
                    stop=chunk_end,
                    skip_group_check=True,
                )

            def drain_group(c, g, banks):
                lg = lgs[c][g]
                off = 0
                for i in banks:
                    if i < 4:
                        nc.vector.tensor_copy(
                            out=lg[:, off : off + 512], in_=attn_ps[i]
                        )
                        off += 512
                    elif psum4:
                        nc.vector.tensor_copy(out=lg[:, off : off + 256], in_=acc8)
                        off += 256
                    else:
                        nc.vector.tensor_copy(
                            out=lg[:, off : off + 256], in_=attn_ps[4][:, 0, :]
                        )
                        off += 256
                nk = len(group_ks(banks))
                # lg free layout: k*256 + h*128 + o  ->  cc_in [h, ci, k, o]
                nc.sync.dma_start(
                    out=cc_ins[c][g][:, :, :, :].rearrange("h p k o -> p k h o"),
                    in_=lg[:, :].rearrange("p (k h o) -> p k h o", k=nk, h=2),
                )
                nc.gpsimd.collective_compute(
                    "ReduceScatter",
                    mybir.AluOpType.add,
                    replica_groups=[[0, 1], [2, 3], [4, 5], [6, 7]],
                    ins=[cc_ins[c][g].ap().opt()],
                    outs=[cc_outs[c][g].ap().opt()],
                )

            def drain_chunk(c):
                for g, banks in enumerate(bank_groups):
                    drain_group(c, g, banks)

            # tail tiles allocated once; reused across reps (WAR-dep serialized)
            parts = [
                [
                    sp.tile(
                        [CIN, len(group_ks(banks)), COH], f32, name=f"part{c}_{g}"
                    )
                    for g, banks in enumerate(bank_groups)
                ]
                for c in range(chunks)
            ]
            ident = sp.tile([128, 128], f32)
            make_identity(nc, ident)
            soft = sp.tile([COH, CIN, 9], f32)
            mx = sp.tile([COH, 1], f32)
            nmx = sp.tile([COH, 1], f32)
            sm = sp.tile([COH, 1], f32)
            rs = sp.tile([COH, 1], f32)

            def tail():
                for c in range(chunks):
                    for g in range(len(bank_groups)):
                        nc.sync.dma_start(
                            out=parts[c][g], in_=cc_outs[c][g][:, :, :]
                        )
                for g in range(len(bank_groups)):
                    for c in range(1, chunks):
                        nc.vector.tensor_add(parts[0][g], parts[0][g], parts[c][g])
                for g, banks in enumerate(bank_groups):
                    for j, kg in enumerate(group_ks(banks)):
                        tp = pc.tile([128, 128], f32, tag="conv")
                        nc.tensor.transpose(
                            out=tp, in_=parts[0][g][:, j, :], identity=ident
                        )
                        nc.vector.tensor_copy(out=soft[:, :, kg], in_=tp)
                # with x16, the 1/sqrt(CIN*9) scale is NOT folded into w1
                # (keeps fp16 weights away from subnormals); apply it here.
                sc = SCALE if x16 else 1.0
                nc.vector.reduce_max(out=mx, in_=soft, axis=mybir.AxisListType.XY)
                nc.scalar.mul(out=nmx, in_=mx, mul=-sc)
                nc.scalar.activation(
                    out=soft,
                    in_=soft,
                    func=mybir.ActivationFunctionType.Exp,
                    bias=nmx,
                    scale=sc,
                    accum_out=sm,
                )
                nc.vector.reciprocal(out=rs, in_=sm)
                nc.vector.tensor_scalar_mul(soft, soft, rs)
                nc.sync.dma_start(out=out_t[:, :, :], in_=soft)

            for rep in range(reps):
                if ring:
                    x3s = {}

                    def load_x3(ti):
                        xtd = mybir.dt.float16 if x16 else f32r
                        xt = xp.tile([CIN, 3, W + 2], xtd, tag="x3", name="x3")
                        rows = 2 if ti == TILES else 3
                        src = x_half[:, 3 * ti : 3 * ti + rows, :]
                        nc.sync.dma_start(
                            out=xt[:, 0:rows, :],
                            in_=src if x16 else src.bitcast(f32r),
                        )
                        x3s[ti] = xt

                    load_x3(0)
                    load_x3(1)
                for t in range(TILES):
                    if psum4 and t in starts:
                        nc.vector.memset(acc8, 0.0)
                    if ring:
                        if t + 2 <= TILES:
                            load_x3(t + 2)
                        cur, nxt = x3s[t], x3s[t + 1]
                    else:
                        xtd = mybir.dt.float16 if x16 else f32r
                        xt = xp.tile([CIN, 5, W + 2], xtd, name="xt")
                        src = x_half[:, 3 * t : 3 * t + 5, :]
                        nc.sync.dma_start(
                            out=xt, in_=src if x16 else src.bitcast(f32r)
                        )
                    yks = []
                    for k in range(9):
                        kh, kw = divmod(k, 3)
                        ps = pc.tile([WP, CH], f32, tag="conv", name="ps")
                        for tap in range(9):
                            dh1, dw1 = divmod(tap, 3)
                            r = kh + dh1
                            s = kw + dw1
                            if ring:
                                src, rr = (cur, r) if r < 3 else (nxt, r - 3)
                                lhsT = src[:, rr, s : s + 3 * WP - 2 : 3]
                            else:
                                lhsT = xt[:, r, s : s + 3 * WP - 2 : 3]
                            nc.tensor.matmul(
                                ps,
                                lhsT,
                                w_sb[:, tap, :],
                                start=(tap == 0),
                                stop=(tap == 8),
                            )
                        yk = yp.tile([WP, CH], f32r, tag="y", name="yk")
                        if act_split and k % act_every == 0:
                            nc.scalar.copy(out=yk, in_=ps)
                        else:
                            nc.vector.tensor_copy(out=yk, in_=ps)
                        yks.append(yk)
                        if k >= lag:
                            attn_mm(k - lag, yks[k - lag], t)
                    for k in range(9 - lag, 9):
                        attn_mm(k, yks[k], t)
                    if (t + 1) in ends:
                        drain_chunk(bounds.index(t + 1))
                tail()
    nc.compile()
    return nc


def _prep_inputs(x, w1, w2, x16=False):
    x = np.ascontiguousarray(np.asarray(x, dtype=np.float32))
    w1 = np.asarray(w1, dtype=np.float32)
    w2 = np.asarray(w2, dtype=np.float32)

    xdt = np.float16 if x16 else np.float32
    wcat = np.empty((CIN, 9, CH), xdt)
    for dh in range(3):
        for dw in range(3):
            tap = dh * 3 + dw
            # with x16 the scale is applied in the softmax exp instead
            wcat[:, tap, :CIN] = w1[:, :, dh, dw].T * (1.0 if x16 else SCALE)
            wcat[:, tap, CIN:] = w2[:, :, dh, dw].T

    xp = np.zeros((B, CIN, H + 2, W + 2), xdt)
    xp[:, :, 1:-1, 1:-1] = x

    in_maps = []
    for c in range(NCORES):
        b, h = divmod(c, 2)
        xh = np.ascontiguousarray(xp[b, :, h * HALF_ROWS : h * HALF_ROWS + 194, :])
        in_maps.append({"x_half": xh, "wcat": wcat})
    return in_maps


class _Runner:
    """Compile once, execute many times with device-resident inputs."""

    def __init__(self, reps=1, **build_kw):
        import jax
        import concourse.mybir as mybir
        from concourse import bass2jax
        from jax.sharding import Mesh, PartitionSpec, NamedSharding
        from jax.experimental.shard_map import shard_map

        self.jax = jax
        nc = _build_nc(reps=reps, **build_kw)
        bass2jax.install_neuronx_cc_hook()

        partition_name = (
            nc.partition_id_tensor.name if nc.partition_id_tensor else None
        )
        in_names, out_names, out_avals, zero_outs = [], [], [], []
        for alloc in nc.m.functions[0].allocations:
            if not isinstance(alloc, mybir.MemoryLocationSet):
                continue
            name = alloc.memorylocations[0].name
            if alloc.kind == "ExternalInput":
                if name != partition_name:
                    in_names.append(name)
            elif alloc.kind == "ExternalOutput":
                out_names.append(name)
                shape = tuple(alloc.tensor_shape)
                dtype = mybir.dt.np(alloc.dtype)
                out_avals.append(jax.core.ShapedArray(shape, dtype))
                zero_outs.append(np.zeros(shape, dtype))
        n_params = len(in_names)
        n_outs = len(out_avals)
        all_names = in_names + out_names
        if partition_name is not None:
            all_names = all_names + [partition_name]

        def _body(*args):
            operands = list(args)
            if partition_name is not None:
                operands.append(bass2jax.partition_id_tensor())
            outs = bass2jax._bass_exec_p.bind(
                *operands,
                out_avals=tuple(out_avals),
                in_names=tuple(all_names),
                out_names=tuple(out_names),
                lowering_input_output_aliases=(),
                sim_require_finite=True,
                sim_require_nnan=True,
                nc=nc,
            )
            return tuple(outs)

        devices = jax.devices()[:NCORES]
        mesh = Mesh(np.asarray(devices), ("core",))
        self.sharded = jax.jit(
            shard_map(
                _body,
                mesh=mesh,
                in_specs=(PartitionSpec("core"),) * (n_params + n_outs),
                out_specs=(PartitionSpec("core"),) * n_outs,
                check_rep=False,
            ),
            keep_unused=True,
        )
        self.sharding = NamedSharding(mesh, PartitionSpec("core"))
        self.in_names = in_names
        self.out_names = out_names
        self.out_avals = out_avals
        self.dev_zeros = [
            jax.device_put(
                np.zeros((NCORES * z.shape[0], *z.shape[1:]), z.dtype), self.sharding
            )
            for z in zero_outs
        ]

    def put_inputs(self, in_maps):
        concat = [
            np.concatenate([np.asarray(m[name]) for m in in_maps], axis=0)
            for name in self.in_names
        ]
        return [self.jax.device_put(a, self.sharding) for a in concat]

    def execute(self, dev_inputs, n=1, block=True):
        for _ in range(n):
            out_arrs = self.sharded(*dev_inputs, *self.dev_zeros)
        if block:
            self.jax.block_until_ready(out_arrs)
        return out_arrs

    def run(self, in_maps):
        out_arrs = self.execute(self.put_inputs(in_maps))
        res = []
        for c in range(NCORES):
            res.append(
                {
                    name: np.asarray(out_arrs[i]).reshape(
                        NCORES, *self.out_avals[i].shape
                    )[c]
                    for i, name in enumerate(self.out_names)
                }
            )
        return res


def get_runner(reps=1, **build_kw):
    key = (reps, tuple(sorted(build_kw.items())))
    if key not in _runners:
        _runners[key] = _Runner(reps=reps, **build_kw)
    return _runners[key]


BEST = dict(chunks=1, ring=True)


def kernel(x, w1, w2):
    in_maps = _prep_inputs(x, w1, w2, x16=BEST.get("x16", False))
    results = get_runner(reps=1, **BEST).run(in_maps)
    out = np.empty((B, COUT, CIN, 9), np.float32)
    for b in range(B):
        out[b] = np.concatenate(
            [results[2 * b]["out"], results[2 * b + 1]["out"]], axis=0
        )
    return out


# revision 3
# speedup vs baseline: 1.0464x; 1.0464x over previous
"""Trainium2 Bass kernel for nn_AttnConvKernel (conv3x3 x2 -> unfold -> gram -> softmax).

Sharding: 8 cores = 4 batch samples x 2 H-halves. Each core computes both convs
for its half in a single fused matmul pass (x window stationary, [W1*scale|W2]
moving, f32r) that directly produces the [positions, channels] layout the
attention contraction needs; logits accumulate in PSUM across the 64 patch-row
tiles.

vs the earlier baseline: (1) x rows are DMA'd once into a 3-row ring buffer
(the 2 halo rows come from the next tile's buffer) instead of 5-row loads —
40% less HBM traffic; (2) the pairwise AllReduce is replaced by a
ReduceScatter with the scatter dim on the cout-halves — half the collective
traffic — and each pair-core transposes/softmaxes only its own 128 couts
(host concatenates); (3) the full body (loop + collective + tail) can be
replayed `reps` times inside one NEFF for robust differential timing."""

import numpy as np

B, CIN, COUT = 4, 128, 256
H = W = 384
WP = W // 3              # 128 patch columns
HALF_ROWS = H // 2       # 192
TILES = HALF_ROWS // 3   # 64 patch-rows per core
CH = CIN + COUT          # 384 fused output channels
COH = COUT // 2          # 128 couts per core in the split tail
NCORES = 8
SCALE = 1.0 / np.sqrt(CIN * 9)

_runners = {}


def _build_nc(reps=1, act_split=True, lag=3, psum4=False, act_every=2,
              chunks=2, ring=True, x_bufs=4, chunk_at=None, tail_split=False,
              x16=False, xph=False):
    import concourse.mybir as mybir
    import concourse.tile as tile
    from concourse import bacc
    from concourse.masks import make_identity

    f32 = mybir.dt.float32
    f32r = mybir.dt.float32r
    xdt = mybir.dt.float16 if x16 else f32

    nc = bacc.Bacc(target_bir_lowering=False, num_devices=NCORES)
    # xph: x stored phase-separated ([.., 3, 130], col 3q+m at [m, q]) so every
    # conv window is a contiguous 128-col slice (the stationary operand of a
    # self-loading f32r matmul otherwise walks SBUF at stride 12B).
    xshape = [CIN, HALF_ROWS + 2, 3, 130] if xph else [CIN, HALF_ROWS + 2, W + 2]
    x_half = nc.dram_tensor("x_half", xshape, xdt, kind="ExternalInput")
    wcat = nc.dram_tensor("wcat", [CIN, 9, CH], xdt, kind="ExternalInput")
    out_t = nc.dram_tensor("out", [COH, CIN, 9], f32, kind="ExternalOutput")
    # bank i in 0..3 holds k=2i,2i+1; bank 4 holds k=8
    if tail_split:
        assert chunks == 1
        bank_groups = [[0, 1], [2, 3, 4]]
    else:
        bank_groups = [[0, 1, 2, 3, 4]]

    def group_ks(banks):
        ks = []
        for i in banks:
            ks += [2 * i, 2 * i + 1] if i < 4 else [8]
        return ks

    cc_ins = [
        [
            nc.dram_tensor(f"cc_in{c}_{g}", [2, CIN, len(group_ks(banks)), COH], f32)
            for g, banks in enumerate(bank_groups)
        ]
        for c in range(chunks)
    ]
    cc_outs = [
        [
            nc.dram_tensor(f"cc_out{c}_{g}", [CIN, len(group_ks(banks)), COH], f32)
            for g, banks in enumerate(bank_groups)
        ]
        for c in range(chunks)
    ]

    if chunk_at is not None:
        assert chunks == 2
        bounds = [chunk_at, TILES]
    else:
        bounds = [round(TILES * (c + 1) / chunks) for c in range(chunks)]
    starts = {0} | {b for b in bounds[:-1]}
    ends = set(bounds)  # tile t is an end if t+1 in ends -> use t+1

    with tile.TileContext(nc) as tc:
        with (
            tc.tile_pool(name="xp", bufs=x_bufs) as xp,
            tc.tile_pool(name="wp", bufs=1) as wp,
            tc.tile_pool(
                name="yp", bufs=(9 if psum4 else 6) if lag <= 2 else lag + 5
            ) as yp,
            tc.tile_pool(name="sp", bufs=1) as sp,
            tc.tile_pool(name="pc", bufs=4 if psum4 else 3, space="PSUM") as pc,
            tc.tile_pool(name="pa", bufs=1, space="PSUM") as pa,
        ):
            wdt = mybir.dt.float16 if x16 else f32r
            w_sb = wp.tile([CIN, 9, CH], wdt)
            nc.sync.dma_start(
                out=w_sb,
                in_=wcat[:, :, :] if x16 else wcat[:, :, :].bitcast(f32r),
            )

            # persistent attn logit accumulators: 2 k's per PSUM bank
            attn_ps = [
                pa.tile([CIN, 2, COUT], f32, tag=f"attn{i}", name=f"attn{i}")
                for i in range(4)
            ]
            if psum4:
                acc8 = sp.tile([CIN, COUT], f32)
            else:
                attn_ps.append(
                    pa.tile([CIN, 1, COUT], f32, tag="attn4", name="attn4")
                )
            lgs = [
                [
                    sp.tile(
                        [CIN, len(group_ks(banks)) * COUT], f32, name=f"lg{c}_{g}"
                    )
                    for g, banks in enumerate(bank_groups)
                ]
                for c in range(chunks)
            ]

            def attn_mm(k, yk, t):
                chunk_start = t in starts
                chunk_end = (t + 1) in ends
                if psum4 and k == 8:
                    ps8 = pc.tile([CIN, COUT], f32, tag="conv", name="ps8")
                    nc.tensor.matmul(
                        ps8, yk[:, 0:CIN], yk[:, CIN:CH], start=True, stop=True
                    )
                    nc.vector.tensor_add(acc8, acc8, ps8)
                    return
                # start=True clears has_written for the WHOLE bank, so only the
                # first k of each 2-k bank may set it (at each chunk start).
                nc.tensor.matmul(
                    attn_ps[k // 2][:, k % 2, :],
                    yk[:, 0:CIN],
                    yk[:, CIN:CH],
                    start=(chunk_start and k % 2 == 0),
                    stop=chunk_end,
                    skip_group_check=True,
                )

            def drain_group(c, g, banks):
                lg = lgs[c][g]
                off = 0
                for i in banks:
                    if i < 4:
                        nc.vector.tensor_copy(
                            out=lg[:, off : off + 512], in_=attn_ps[i]
                        )
                        off += 512
                    elif psum4:
                        nc.vector.tensor_copy(out=lg[:, off : off + 256], in_=acc8)
                        off += 256
                    else:
                        nc.vector.tensor_copy(
                            out=lg[:, off : off + 256], in_=attn_ps[4][:, 0, :]
                        )
                        off += 256
                nk = len(group_ks(banks))
                # lg free layout: k*256 + h*128 + o  ->  cc_in [h, ci, k, o]
                nc.sync.dma_start(
                    out=cc_ins[c][g][:, :, :, :].rearrange("h p k o -> p k h o"),
                    in_=lg[:, :].rearrange("p (k h o) -> p k h o", k=nk, h=2),
                )
                nc.gpsimd.collective_compute(
                    "ReduceScatter",
                    mybir.AluOpType.add,
                    replica_groups=[[0, 1], [2, 3], [4, 5], [6, 7]],
                    ins=[cc_ins[c][g].ap().opt()],
                    outs=[cc_outs[c][g].ap().opt()],
                )

            def drain_chunk(c):
                for g, banks in enumerate(bank_groups):
                    drain_group(c, g, banks)

            # tail tiles allocated once; reused across reps (WAR-dep serialized)
            parts = [
                [
                    sp.tile(
                        [CIN, len(group_ks(banks)), COH], f32, name=f"part{c}_{g}"
                    )
                    for g, banks in enumerate(bank_groups)
                ]
                for c in range(chunks)
            ]
            ident = sp.tile([128, 128], f32)
            make_identity(nc, ident)
            soft = sp.tile([COH, CIN, 9], f32)
            mx = sp.tile([COH, 1], f32)
            nmx = sp.tile([COH, 1], f32)
            sm = sp.tile([COH, 1], f32)
            rs = sp.tile([COH, 1], f32)

            def tail():
                for c in range(chunks):
                    for g in range(len(bank_groups)):
                        nc.sync.dma_start(
                            out=parts[c][g], in_=cc_outs[c][g][:, :, :]
                        )
                for g in range(len(bank_groups)):
                    for c in range(1, chunks):
                        nc.vector.tensor_add(parts[0][g], parts[0][g], parts[c][g])
                for g, banks in enumerate(bank_groups):
                    for j, kg in enumerate(group_ks(banks)):
                        tp = pc.tile([128, 128], f32, tag="conv")
                        nc.tensor.transpose(
                            out=tp, in_=parts[0][g][:, j, :], identity=ident
                        )
                        nc.vector.tensor_copy(out=soft[:, :, kg], in_=tp)
                # with x16, the 1/sqrt(CIN*9) scale is NOT folded into w1
                # (keeps fp16 weights away from subnormals); apply it here.
                sc = SCALE if x16 else 1.0
                nc.vector.reduce_max(out=mx, in_=soft, axis=mybir.AxisListType.XY)
                nc.scalar.mul(out=nmx, in_=mx, mul=-sc)
                nc.scalar.activation(
                    out=soft,
                    in_=soft,
                    func=mybir.ActivationFunctionType.Exp,
                    bias=nmx,
                    scale=sc,
                    accum_out=sm,
                )
                nc.vector.reciprocal(out=rs, in_=sm)
                nc.vector.tensor_scalar_mul(soft, soft, rs)
                nc.sync.dma_start(out=out_t[:, :, :], in_=soft)

            for rep in range(reps):
                if ring:
                    x3s = {}

                    def load_x3(ti):
                        xtd = mybir.dt.float16 if x16 else f32r
                        shape = [CIN, 3, 3, 130] if xph else [CIN, 3, W + 2]
                        xt = xp.tile(shape, xtd, tag="x3", name="x3")
                        rows = 2 if ti == TILES else 3
                        src = x_half[:, 3 * ti : 3 * ti + rows]
                        nc.sync.dma_start(
                            out=xt[:, 0:rows],
                            in_=src if x16 else src.bitcast(f32r),
                        )
                        x3s[ti] = xt

                    load_x3(0)
                    load_x3(1)
                for t in range(TILES):
                    if psum4 and t in starts:
                        nc.vector.memset(acc8, 0.0)
                    if ring:
                        if t + 2 <= TILES:
                            load_x3(t + 2)
                        cur, nxt = x3s[t], x3s[t + 1]
                    else:
                        xtd = mybir.dt.float16 if x16 else f32r
                        xt = xp.tile([CIN, 5, W + 2], xtd, name="xt")
                        src = x_half[:, 3 * t : 3 * t + 5, :]
                        nc.sync.dma_start(
                            out=xt, in_=src if x16 else src.bitcast(f32r)
                        )
                    yks = []
                    for k in range(9):
                        kh, kw = divmod(k, 3)
                        ps = pc.tile([WP, CH], f32, tag="conv", name="ps")
                        for tap in range(9):
                            dh1, dw1 = divmod(tap, 3)
                            r = kh + dh1
                            s = kw + dw1
                            if ring:
                                src, rr = (cur, r) if r < 3 else (nxt, r - 3)
                                if xph:
                                    lhsT = src[:, rr, s % 3, s // 3 : s // 3 + WP]
                                else:
                                    lhsT = src[:, rr, s : s + 3 * WP - 2 : 3]
                            else:
                                lhsT = xt[:, r, s : s + 3 * WP - 2 : 3]
                            nc.tensor.matmul(
                                ps,
                                lhsT,
                                w_sb[:, tap, :],
                                start=(tap == 0),
                                stop=(tap == 8),
                            )
                        yk = yp.tile([WP, CH], f32r, tag="y", name="yk")
                        if act_split and k % act_every == 0:
                            nc.scalar.copy(out=yk, in_=ps)
                        else:
                            nc.vector.tensor_copy(out=yk, in_=ps)
                        yks.append(yk)
                        if k >= lag:
                            attn_mm(k - lag, yks[k - lag], t)
                    for k in range(9 - lag, 9):
                        attn_mm(k, yks[k], t)
                    if (t + 1) in ends:
                        drain_chunk(bounds.index(t + 1))
                tail()
    nc.compile()
    return nc


def _prep_inputs(x, w1, w2, x16=False, xph=False):
    x = np.ascontiguousarray(np.asarray(x, dtype=np.float32))
    w1 = np.asarray(w1, dtype=np.float32)
    w2 = np.asarray(w2, dtype=np.float32)

    xdt = np.float16 if x16 else np.float32
    wcat = np.empty((CIN, 9, CH), xdt)
    for dh in range(3):
        for dw in range(3):
            tap = dh * 3 + dw
            # with x16 the scale is applied in the softmax exp instead
            wcat[:, tap, :CIN] = w1[:, :, dh, dw].T * (1.0 if x16 else SCALE)
            wcat[:, tap, CIN:] = w2[:, :, dh, dw].T

    xp = np.zeros((B, CIN, H + 2, W + 2), xdt)
    xp[:, :, 1:-1, 1:-1] = x

    in_maps = []
    for c in range(NCORES):
        b, h = divmod(c, 2)
        xh = np.ascontiguousarray(xp[b, :, h * HALF_ROWS : h * HALF_ROWS + 194, :])
        if xph:
            xq = np.zeros((CIN, 194, 3, 130), xdt)
            for m in range(3):
                ncols = (W + 2 - m + 2) // 3
                xq[:, :, m, :ncols] = xh[:, :, m::3]
            xh = xq
        in_maps.append({"x_half": xh, "wcat": wcat})
    return in_maps


class _Runner:
    """Compile once, execute many times with device-resident inputs."""

    def __init__(self, reps=1, **build_kw):
        import jax
        import concourse.mybir as mybir
        from concourse import bass2jax
        from jax.sharding import Mesh, PartitionSpec, NamedSharding
        from jax.experimental.shard_map import shard_map

        self.jax = jax
        nc = _build_nc(reps=reps, **build_kw)
        bass2jax.install_neuronx_cc_hook()

        partition_name = (
            nc.partition_id_tensor.name if nc.partition_id_tensor else None
        )
        in_names, out_names, out_avals, zero_outs = [], [], [], []
        for alloc in nc.m.functions[0].allocations:
            if not isinstance(alloc, mybir.MemoryLocationSet):
                continue
            name = alloc.memorylocations[0].name
            if alloc.kind == "ExternalInput":
                if name != partition_name:
                    in_names.append(name)
            elif alloc.kind == "ExternalOutput":
                out_names.append(name)
                shape = tuple(alloc.tensor_shape)
                dtype = mybir.dt.np(alloc.dtype)
                out_avals.append(jax.core.ShapedArray(shape, dtype))
                zero_outs.append(np.zeros(shape, dtype))
        n_params = len(in_names)
        n_outs = len(out_avals)
        all_names = in_names + out_names
        if partition_name is not None:
            all_names = all_names + [partition_name]

        def _body(*args):
            operands = list(args)
            if partition_name is not None:
                operands.append(bass2jax.partition_id_tensor())
            outs = bass2jax._bass_exec_p.bind(
                *operands,
                out_avals=tuple(out_avals),
                in_names=tuple(all_names),
                out_names=tuple(out_names),
                lowering_input_output_aliases=(),
                sim_require_finite=True,
                sim_require_nnan=True,
                nc=nc,
            )
            return tuple(outs)

        devices = jax.devices()[:NCORES]
        mesh = Mesh(np.asarray(devices), ("core",))
        self.sharded = jax.jit(
            shard_map(
                _body,
                mesh=mesh,
                in_specs=(PartitionSpec("core"),) * (n_params + n_outs),
                out_specs=(PartitionSpec("core"),) * n_outs,
                check_rep=False,
            ),
            keep_unused=True,
        )
        self.sharding = NamedSharding(mesh, PartitionSpec("core"))
        self.in_names = in_names
        self.out_names = out_names
        self.out_avals = out_avals
        self.dev_zeros = [
            jax.device_put(
                np.zeros((NCORES * z.shape[0], *z.shape[1:]), z.dtype), self.sharding
            )
            for z in zero_outs
        ]

    def put_inputs(self, in_maps):
        concat = [
            np.concatenate([np.asarray(m[name]) for m in in_maps], axis=0)
            for name in self.in_names
        ]
        return [self.jax.device_put(a, self.sharding) for a in concat]

    def execute(self, dev_inputs, n=1, block=True):
        for _ in range(n):
            out_arrs = self.sharded(*dev_inputs, *self.dev_zeros)
        if block:
            self.jax.block_until_ready(out_arrs)
        return out_arrs

    def run(self, in_maps):
        out_arrs = self.execute(self.put_inputs(in_maps))
        res = []
        for c in range(NCORES):
            res.append(
                {
                    name: np.asarray(out_arrs[i]).reshape(
                        NCORES, *self.out_avals[i].shape
                    )[c]
                    for i, name in enumerate(self.out_names)
                }
            )
        return res


def get_runner(reps=1, **build_kw):
    key = (reps, tuple(sorted(build_kw.items())))
    if key not in _runners:
        _runners[key] = _Runner(reps=reps, **build_kw)
    return _runners[key]


BEST = dict(chunks=1, ring=True, xph=True)


def kernel(x, w1, w2):
    in_maps = _prep_inputs(
        x, w1, w2, x16=BEST.get("x16", False), xph=BEST.get("xph", False)
    )
    results = get_runner(reps=1, **BEST).run(in_maps)
    out = np.empty((B, COUT, CIN, 9), np.float32)
    for b in range(B):
        out[b] = np.concatenate(
            [results[2 * b]["out"], results[2 * b + 1]["out"]], axis=0
        )
    return out
